# revision 2
# baseline (speedup 1.0000x reference)
"""CovaBlock kernel for 8 trn2 NeuronCores.

reference computation:
  cova[w] = covariance of support class w over its 8*32*32 = 8192 pixels  [16,128,128]
  qn[b]   = x1[b] with each channel row L2-normalized over the 1024 pixels
  sim[b, w, i] = qn[b,:,i]^T @ cova[w] @ qn[b,:,i]   -> [256, 16*1024]

Plan:
  launch 1: shard the 16 classes over 8 cores (2 each) -> cova on device
  host:     cholesky(cova) = L L^T  (tiny, 11 MFLOP)
  launch 2: shard the 256 queries over 8 cores (32 each).
    For classes assigned to ACT: U = L_w^T qn (PE, f32r), V = U^2 (ACT square)
    For classes assigned to DVE: U = C_w qn  (PE, f32r), V = U * qn (DVE mult)
    sim rows = ones^T V (PE ones-matmuls, 4 concurrent via col tile_position)
"""

import os

import numpy as np

import concourse.bass as bass
import concourse.tile as tile
from concourse import bacc, mybir
from concourse.bass_utils import run_bass_kernel_spmd
from concourse.masks import make_identity

F32 = mybir.dt.float32
F32R = mybir.dt.float32r
BF16 = mybir.dt.bfloat16

N_CORES = 8
B, C, HW = 256, 128, 1024          # x1: [B, C, 32, 32]
W, S = 16, 8                       # x2: [W, S, C, 32, 32]
NS = S * HW                        # samples per class = 8192
BS = B // N_CORES                  # 32 queries per core
WS = W // N_CORES                  # 2 classes per core

# classes 0..ACT_W-1 take the cholesky/square path (ACT engine),
# classes ACT_W..15 take the direct/multiply path (DVE engine)
ACT_W = 9

_CACHE = {}


def _build_cova_nc():
    """Per-core: x2 pair [2, 8, 128, 1024] f32 -> cova pair [2, 128, 128] f32."""
    nc = bacc.Bacc("TRN2", target_bir_lowering=False, debug=False,
                   num_devices=N_CORES)
    x2p = nc.dram_tensor("x2p", [WS, S, C, HW], F32, kind="ExternalInput").ap()
    cov = nc.dram_tensor("cova_pair", [WS, C, C], F32, kind="ExternalOutput").ap()

    inv_nm1 = 1.0 / (NS - 1)
    # mean scale so that mmT comes out as N/(N-1) * m m^T directly:
    # m'' = msum * sqrt(N/(N-1)) / N
    mscale = float(np.sqrt(NS / (NS - 1.0)) / NS)

    with tile.TileContext(nc) as tc:
        with (
            tc.tile_pool(name="consts", bufs=1) as consts,
            tc.tile_pool(name="raw", bufs=3) as raw,
            tc.tile_pool(name="xb", bufs=9) as xbp,
            tc.tile_pool(name="xt", bufs=3) as xtp,
            tc.tile_pool(name="small", bufs=4) as small,
            tc.tile_pool(name="scratch", bufs=2) as scratch,
            tc.tile_pool(name="cout", bufs=2) as cout,
            tc.tile_pool(name="pt", bufs=3, space="PSUM") as pt,
            tc.tile_pool(name="pe", bufs=2, space="PSUM") as pe,
            tc.tile_pool(name="pm", bufs=1, space="PSUM") as pm,
            tc.tile_pool(name="pmr", bufs=1, space="PSUM") as pmr,
        ):
            ident = consts.tile([128, 128], BF16)
            make_identity(nc, ident)
            ident32 = consts.tile([128, 128], F32)
            make_identity(nc, ident32)

            for w in range(WS):
                # load shots, convert to bf16 on ACT while accumulating the
                # per-channel sum into msum columns
                xb16 = []
                msum = small.tile([C, S], F32)
                for s in range(S):
                    xr = raw.tile([C, HW], F32, tag="raw")
                    nc.default_dma_engine.dma_start(out=xr, in_=x2p[w, s])
                    xb = xbp.tile([C, HW], BF16, tag=f"xb{s}")
                    nc.scalar.activation(xb, xr, mybir.ActivationFunctionType.Copy,
                                         accum_out=msum[:, s : s + 1])
                    xb16.append(xb)

                # m'' = (sum_s msum[:, s]) * mscale, as fp32 [C, 1]
                mtot = small.tile([C, 1], F32)
                nc.vector.reduce_sum(mtot, msum, axis=mybir.AxisListType.X)
                mpp = small.tile([C, 1], F32)
                nc.scalar.mul(mpp, mtot, mscale)

                # E = sum over 64 chunks of X_chunk X_chunk^T  (bf16, fp32 acc)
                e_ps = pe.tile([C, C], F32, tag="E")
                for s in range(S):
                    for k in range(HW // 128):
                        chunk = xb16[s][:, k * 128 : (k + 1) * 128]
                        xt_ps = pt.tile([128, C], BF16, tag="xt_ps")
                        nc.tensor.transpose(xt_ps, chunk, ident)
                        xt = xtp.tile([128, C], BF16, tag="xt")
                        nc.vector.tensor_copy(xt, xt_ps)
                        nc.tensor.matmul(e_ps, xt, xt,
                                         start=(s == 0 and k == 0),
                                         stop=(s == S - 1 and k == HW // 128 - 1))

                # mmT = m'' m''^T via K=1 matmul; needs m'' as a [1, C] row
                mrow_ps = pmr.tile([1, C], F32, tag="mrow")
                nc.tensor.transpose(mrow_ps, mpp, ident32)
                mrow = small.tile([1, C], F32)
                nc.vector.tensor_copy(mrow, mrow_ps)
                mmT_ps = pm.tile([C, C], F32, tag="mmT")
                nc.tensor.matmul(mmT_ps, mrow, mrow, start=True, stop=True)
                mmT = scratch.tile([C, C], F32, tag="mmT_sb")
                nc.vector.tensor_copy(mmT, mmT_ps)

                # C_w = E * 1/(N-1) - mmT
                cw = cout.tile([C, C], F32, tag="cw")
                nc.vector.scalar_tensor_tensor(
                    out=cw, in0=e_ps, scalar=inv_nm1, in1=mmT,
                    op0=mybir.AluOpType.mult, op1=mybir.AluOpType.subtract)
                nc.default_dma_engine.dma_start(out=cov[w], in_=cw)

    nc.compile()
    return nc


def _build_sim_nc():
    """Per-core: x1 shard [32, 128, 1024] + chol/cova [16, 128, 128] ->
    sim shard [32, 16 * 1024]."""
    nc = bacc.Bacc("TRN2", target_bir_lowering=False, debug=False,
                   num_devices=N_CORES)
    x1s = nc.dram_tensor("x1s", [BS, C, HW], F32, kind="ExternalInput").ap()
    chol = nc.dram_tensor("chol", [W, C, C], F32, kind="ExternalInput").ap()
    cova = nc.dram_tensor("cova", [W, C, C], F32, kind="ExternalInput").ap()
    out = nc.dram_tensor("sim", [BS, W, HW], F32, kind="ExternalOutput").ap()

    with tile.TileContext(nc) as tc:
        with (
            tc.tile_pool(name="consts", bufs=1) as consts,
            tc.tile_pool(name="mats", bufs=1) as mats,
            tc.tile_pool(name="xb", bufs=3) as xbp,
            tc.tile_pool(name="qn", bufs=3) as qnp,
            tc.tile_pool(name="sq", bufs=2) as sqp,
            tc.tile_pool(name="vv", bufs=6) as vvp,
            tc.tile_pool(name="st", bufs=3) as stp,
            tc.tile_pool(name="small", bufs=6) as small,
            tc.tile_pool(name="pu", bufs=2, space="PSUM") as pu,
            tc.tile_pool(name="pr", bufs=2, space="PSUM") as pr,
        ):
            # M=32 all-ones stationary operand: every column of a col-group
            # strip computes the same k-sum, so the whole [128, n] psum tile is
            # valid and a stride-1 copy can stage it for DMA (engines cannot
            # read strided partitions; DMA can).
            ones_r = consts.tile([C, 32], BF16)
            nc.vector.memset(ones_r, 1.0)

            # stationary matrices: L for ACT classes, C for DVE classes.
            # f32r consumers require a rounded producer, so convert after DMA.
            mat_raw = mats.tile([C, W, C], F32)
            for w in range(W):
                src = chol if w < ACT_W else cova
                nc.default_dma_engine.dma_start(out=mat_raw[:, w, :], in_=src[w])
            mat = mats.tile([C, W, C], F32R)
            nc.vector.tensor_copy(mat, mat_raw)

            for b in range(BS):
                xb = xbp.tile([C, HW], F32, tag="xb")
                nc.default_dma_engine.dma_start(out=xb, in_=x1s[b])

                # channel norms: n2 = sum_i x^2, rinv = 1/sqrt(n2)
                sq = sqp.tile([C, HW], F32, tag="sq")
                n2 = small.tile([C, 1], F32, tag="n2")
                nc.scalar.activation(sq, xb, mybir.ActivationFunctionType.Square,
                                     accum_out=n2)
                nrm = small.tile([C, 1], F32, tag="nrm")
                nc.scalar.sqrt(nrm, n2)
                rinv = small.tile([C, 1], F32, tag="rinv")
                nc.vector.reciprocal(rinv, nrm)
                qn_r = qnp.tile([C, HW], F32R, tag="qn")
                nc.vector.tensor_scalar_mul(qn_r, xb, rinv)
                qn = qn_r.bitcast(F32)

                vtiles = []
                for w in range(W):
                    u_ps = pu.tile([C, HW], F32, tag="u")
                    lw = mat[:, w, :]
                    for h in range(2):
                        cols = slice(h * 512, (h + 1) * 512)
                        nc.tensor.matmul(u_ps[:, cols], lw, qn_r[:, cols],
                                         start=True, stop=True)
                    v = vvp.tile([C, HW], BF16, tag="v")
                    if w < ACT_W:
                        # cholesky path: V = U^2
                        nc.scalar.square(v, u_ps)
                    else:
                        # direct path: V = U * qn
                        nc.vector.tensor_mul(v, u_ps, qn)
                    vtiles.append(v)

                    if (w + 1) % 4 == 0:
                        r = w // 4
                        red = pr.tile([C, HW], F32, tag="red")
                        for j in range(4):
                            vr = vtiles[4 * r + j]
                            for h in range(2):
                                cols = slice(h * 512, (h + 1) * 512)
                                nc.tensor.matmul(red[32 * j : 32 * j + 32, cols],
                                                 ones_r, vr[:, cols],
                                                 start=True, stop=True,
                                                 tile_position=(0, 32 * j))
                        stage = stp.tile([C, HW], F32, tag="stage")
                        if r % 2 == 0:
                            nc.scalar.copy(stage, red)
                        else:
                            nc.vector.tensor_copy(stage, red)
                        rows = stage.rearrange("(j p) n -> j p n", p=32)[:, 0, :]
                        nc.default_dma_engine.dma_start(
                            out=out[b, 4 * r : 4 * r + 4, :], in_=rows)

    nc.compile()
    return nc


def kernel(x1: np.ndarray, x2: np.ndarray) -> np.ndarray:
    x1 = np.ascontiguousarray(np.asarray(x1, dtype=np.float32)).reshape(B, C, HW)
    x2 = np.ascontiguousarray(np.asarray(x2, dtype=np.float32)).reshape(W, S, C, HW)
    core_ids = list(range(N_CORES))

    profile = bool(os.environ.get("COVA_PROFILE"))
    kw1, kw2 = {}, {}
    if profile:
        import shutil, tempfile
        for d in ("/tmp/cova_prof1", "/tmp/cova_prof2"):
            shutil.rmtree(d, ignore_errors=True)
            os.makedirs(d)
        kw1 = dict(trace=True, tmpdir="/tmp/cova_prof1")
        kw2 = dict(trace=True, tmpdir="/tmp/cova_prof2")

    if "cova" not in _CACHE:
        _CACHE["cova"] = _build_cova_nc()
    cova_in = [{"x2p": np.ascontiguousarray(x2[WS * k : WS * (k + 1)])}
               for k in range(N_CORES)]
    res1 = run_bass_kernel_spmd(_CACHE["cova"], cova_in, core_ids, **kw1)
    cova = np.concatenate([res1.results[k]["cova_pair"] for k in range(N_CORES)], 0)

    chol = np.linalg.cholesky(cova.astype(np.float64)).astype(np.float32)
    chol = np.ascontiguousarray(chol)
    cova = np.ascontiguousarray(cova)

    if "sim" not in _CACHE:
        _CACHE["sim"] = _build_sim_nc()
    sim_in = [{"x1s": np.ascontiguousarray(x1[BS * k : BS * (k + 1)]),
               "chol": chol, "cova": cova} for k in range(N_CORES)]
    res2 = run_bass_kernel_spmd(_CACHE["sim"], sim_in, core_ids, **kw2)
    if profile:
        _CACHE["exec_ns"] = (res1.exec_time_ns, res2.exec_time_ns)
    sim = np.concatenate([res2.results[k]["sim"] for k in range(N_CORES)], 0)
    return sim.reshape(B, W * HW)



# revision 11
# speedup vs baseline: 1.0980x; 1.0980x over previous
"""CovaBlock kernel for 8 trn2 NeuronCores.

reference computation:
  cova[w] = covariance of support class w over its 8*32*32 = 8192 pixels  [16,128,128]
  qn[b]   = x1[b] with each channel row L2-normalized over the 1024 pixels
  sim[b, w, i] = qn[b,:,i]^T @ cova[w] @ qn[b,:,i]   -> [256, 16*1024]

Plan:
  launch 1: shard the 16 classes over 8 cores (2 each) -> cova on device
  host:     cholesky(cova) = L L^T  (tiny)
  launch 2: shard the 256 queries over 8 cores (32 each).
    For ACT-path classes: U = L_w^T qn (PE bf16), V = U^2 (ACT square)
    For DVE-path classes: U = C_w qn  (PE bf16), V = U * qn (DVE mult)
    sim rows = ones^T V (PE ones-matmuls, packed 4 classes/psum tile via
    col tile_position), DMA'd straight from PSUM (no stage copies).
"""

import os

import numpy as np

import concourse.bass as bass
import concourse.tile as tile
from concourse import bacc, mybir
from concourse.bass_utils import run_bass_kernel_spmd
from concourse.masks import make_identity

F32 = mybir.dt.float32
F32R = mybir.dt.float32r
BF16 = mybir.dt.bfloat16

N_CORES = 8
B, C, HW = 256, 128, 1024          # x1: [B, C, 32, 32]
W, S = 16, 8                       # x2: [W, S, C, 32, 32]
NS = S * HW                        # samples per class = 8192
BS = B // N_CORES                  # 32 queries per core
WS = W // N_CORES                  # 2 classes per core

# classes 0..ACT_W-1 take the cholesky/square path (ACT engine),
# classes ACT_W..15 take the direct/multiply path (DVE engine)
ACT_W = 8

# who stages each quad's reduce psum tile to SBUF for the output DMA
# (GPSIMD/Pool cannot access PSUM — birverifier rejects it)
STAGE_ENGINES = ("act", "act", "act", "dve")

_CACHE = {}


def _build_cova_nc():
    """Per-core: x2 pair [2, 8, 128, 1024] f32 -> cova pair [2, 128, 128] f32."""
    nc = bacc.Bacc("TRN2", target_bir_lowering=False, debug=False,
                   num_devices=N_CORES)
    x2p = nc.dram_tensor("x2p", [WS, S, C, HW], F32, kind="ExternalInput").ap()
    cov = nc.dram_tensor("cova_pair", [WS, C, C], F32, kind="ExternalOutput").ap()

    inv_nm1 = 1.0 / (NS - 1)
    # mean scale so that mmT comes out as N/(N-1) * m m^T directly:
    # m'' = msum * sqrt(N/(N-1)) / N
    mscale = float(np.sqrt(NS / (NS - 1.0)) / NS)

    with tile.TileContext(nc) as tc:
        with (
            tc.tile_pool(name="consts", bufs=1) as consts,
            tc.tile_pool(name="raw", bufs=4) as raw,
            tc.tile_pool(name="xb", bufs=9) as xbp,
            tc.tile_pool(name="xt", bufs=4) as xtp,
            tc.tile_pool(name="small", bufs=4) as small,
            tc.tile_pool(name="scratch", bufs=2) as scratch,
            tc.tile_pool(name="cout", bufs=2) as cout,
            tc.tile_pool(name="pt", bufs=2, space="PSUM") as pt,
            tc.tile_pool(name="pe", bufs=2, space="PSUM") as pe,
            tc.tile_pool(name="pm", bufs=1, space="PSUM") as pm,
            tc.tile_pool(name="pmr", bufs=1, space="PSUM") as pmr,
        ):
            ident = consts.tile([128, 128], BF16)
            make_identity(nc, ident)
            ident32 = consts.tile([128, 128], F32)
            make_identity(nc, ident32)

            for w in range(WS):
                # load shots, convert to bf16 on ACT while accumulating the
                # per-channel sum into msum columns
                xb16 = []
                msum = small.tile([C, S], F32)
                for s in range(S):
                    xr = raw.tile([C, HW], F32, tag="raw")
                    nc.default_dma_engine.dma_start(out=xr, in_=x2p[w, s])
                    xb = xbp.tile([C, HW], BF16, tag=f"xb{s}")
                    nc.scalar.activation(xb, xr, mybir.ActivationFunctionType.Copy,
                                         accum_out=msum[:, s : s + 1])
                    xb16.append(xb)

                # m'' = (sum_s msum[:, s]) * mscale, as fp32 [C, 1]
                mtot = small.tile([C, 1], F32)
                nc.vector.reduce_sum(mtot, msum, axis=mybir.AxisListType.X)
                mpp = small.tile([C, 1], F32)
                nc.scalar.mul(mpp, mtot, mscale)

                # E = sum over 64 chunks of X_chunk X_chunk^T  (bf16, fp32 acc).
                # Transposes land 4-wide in a [128, 512] psum tile so the
                # psum->sbuf copy is one wide op instead of four narrow ones;
                # copies alternate DVE/ACT to balance the engines.
                e_ps = pe.tile([C, C], F32, tag="E")
                nchunk = HW // 128          # 8 chunks of 128 per shot
                for s in range(S):
                    for q in range(2):      # two [128, 512] transpose groups
                        xt_ps = pt.tile([128, 512], BF16, tag="xt_ps")
                        for k in range(4):
                            chunk = xb16[s][:, (4 * q + k) * 128 : (4 * q + k + 1) * 128]
                            nc.tensor.transpose(
                                xt_ps[:, k * 128 : (k + 1) * 128], chunk, ident)
                        xt = xtp.tile([128, 512], BF16, tag="xt")
                        if (s + q) % 2 == 0:
                            nc.vector.tensor_copy(xt, xt_ps)
                        else:
                            nc.scalar.copy(xt, xt_ps)
                        for k in range(4):
                            xk = xt[:, k * 128 : (k + 1) * 128]
                            first = s == 0 and q == 0 and k == 0
                            last = (s == S - 1 and q == 1 and k == 3)
                            nc.tensor.matmul(e_ps, xk, xk,
                                             start=first, stop=last)

                # mmT = m'' m''^T via K=1 matmul; needs m'' as a [1, C] row
                mrow_ps = pmr.tile([1, C], F32, tag="mrow")
                nc.tensor.transpose(mrow_ps, mpp, ident32)
                mrow = small.tile([1, C], F32)
                nc.vector.tensor_copy(mrow, mrow_ps)
                mmT_ps = pm.tile([C, C], F32, tag="mmT")
                nc.tensor.matmul(mmT_ps, mrow, mrow, start=True, stop=True)
                mmT = scratch.tile([C, C], F32, tag="mmT_sb")
                nc.vector.tensor_copy(mmT, mmT_ps)

                # C_w = E * 1/(N-1) - mmT
                cw = cout.tile([C, C], F32, tag="cw")
                nc.vector.scalar_tensor_tensor(
                    out=cw, in0=e_ps, scalar=inv_nm1, in1=mmT,
                    op0=mybir.AluOpType.mult, op1=mybir.AluOpType.subtract)
                nc.default_dma_engine.dma_start(out=cov[w], in_=cw)

    nc.compile()
    return nc


def _build_sim_nc():
    """Per-core: x1 shard [32, 128, 1024] + mats [16, 128, 128] ->
    sim shard [32, 16 * 1024].

    mats[w] = chol(cova[w]) for w < ACT_W else cova[w], prepacked on host.
    """
    nc = bacc.Bacc("TRN2", target_bir_lowering=False, debug=False,
                   num_devices=N_CORES)
    x1s = nc.dram_tensor("x1s", [BS, C, HW], BF16, kind="ExternalInput").ap()
    mats = nc.dram_tensor("mats", [W, C, C], F32, kind="ExternalInput").ap()
    out = nc.dram_tensor("sim", [BS, W, HW], F32, kind="ExternalOutput").ap()

    HC = 512  # matmul moving-dim chunk

    with tile.TileContext(nc) as tc:
        with (
            tc.tile_pool(name="consts", bufs=1) as consts,
            tc.tile_pool(name="mats", bufs=1) as matp,
            tc.tile_pool(name="xb", bufs=4) as xbp,
            tc.tile_pool(name="sq", bufs=2) as sqp,
            tc.tile_pool(name="qn", bufs=3) as qnp,
            tc.tile_pool(name="vv", bufs=8) as vvp,
            tc.tile_pool(name="st", bufs=3) as stp,
            tc.tile_pool(name="small", bufs=8) as small,
            tc.tile_pool(name="pu", bufs=4, space="PSUM") as pu,
            tc.tile_pool(name="pr", bufs=2, space="PSUM") as pr,
        ):
            # M=32 all-ones stationary: every column of a col-group strip
            # computes the same k-sum; row j*32 of the strip carries class j's
            # sums and DMA reads it straight from PSUM.
            ones_r = consts.tile([C, 32], BF16)
            nc.vector.memset(ones_r, 1.0)

            # stationary matrices in bf16 (1 cycle/row at any p-state)
            mat_raw = matp.tile([C, W, C], F32)
            for w in range(W):
                nc.default_dma_engine.dma_start(out=mat_raw[:, w, :], in_=mats[w])
            mat = matp.tile([C, W, C], BF16)
            nc.vector.tensor_copy(mat, mat_raw)

            for b in range(BS):
                xb = xbp.tile([C, HW], BF16, tag="xb")
                nc.default_dma_engine.dma_start(out=xb, in_=x1s[b])

                # channel norms: n2 = sum_i x^2, rinv = 1/sqrt(n2)
                sq = sqp.tile([C, HW], BF16, tag="sq")
                n2 = small.tile([C, 1], F32, tag="n2")
                nc.scalar.activation(sq, xb, mybir.ActivationFunctionType.Square,
                                     accum_out=n2)
                nrm = small.tile([C, 1], F32, tag="nrm")
                nc.scalar.sqrt(nrm, n2)
                rinv = small.tile([C, 1], F32, tag="rinv")
                nc.vector.reciprocal(rinv, nrm)
                # all-bf16 SBUF operands -> DVE 4x mode
                qn = qnp.tile([C, HW], BF16, tag="qn")
                nc.vector.tensor_scalar_mul(qn, xb, rinv)

                for g in range(4):
                    red = pr.tile([C, HW], F32, tag="red")
                    # projections + V for the whole quad first, then the
                    # reduces: by the time red(j) runs, v(j) has had 3+
                    # classes of slack, so the PE never waits on ACT/DVE.
                    vtiles = []
                    for j in range(4):
                        w = 4 * g + j
                        lw = mat[:, w, :]
                        v = vvp.tile([C, HW], BF16, tag="v")
                        for h in range(2):
                            cols = slice(h * HC, (h + 1) * HC)
                            u_ps = pu.tile([C, HC], F32, tag="u")
                            nc.tensor.matmul(u_ps, lw, qn[:, cols],
                                             start=True, stop=True)
                            if w < ACT_W:
                                # cholesky path: V = U^2
                                nc.scalar.square(v[:, cols], u_ps)
                            else:
                                # direct path: V = U * qn
                                nc.vector.tensor_mul(v[:, cols], u_ps, qn[:, cols])
                        vtiles.append(v)
                    for j in range(4):
                        for h in range(2):
                            cols = slice(h * HC, (h + 1) * HC)
                            nc.tensor.matmul(red[32 * j : 32 * j + 32, cols],
                                             ones_r, vtiles[j][:, cols],
                                             start=True, stop=True,
                                             tile_position=(0, 32 * j))
                    stage = stp.tile([C, HW], F32, tag="stage")
                    if STAGE_ENGINES[g] == "act":
                        nc.scalar.copy(stage, red)
                    else:
                        nc.vector.tensor_copy(stage, red)
                    srows = stage.rearrange("(j p) n -> j p n", p=32)[:, 0, :]
                    nc.default_dma_engine.dma_start(
                        out=out[b, 4 * g : 4 * g + 4, :], in_=srows)

    nc.compile()
    return nc


def kernel(x1: np.ndarray, x2: np.ndarray) -> np.ndarray:
    import ml_dtypes
    x1 = np.ascontiguousarray(
        np.asarray(x1, dtype=np.float32).astype(ml_dtypes.bfloat16)
    ).reshape(B, C, HW)
    x2 = np.ascontiguousarray(np.asarray(x2, dtype=np.float32)).reshape(W, S, C, HW)
    core_ids = list(range(N_CORES))

    profile = bool(os.environ.get("COVA_PROFILE"))
    kw1, kw2 = {}, {}
    if profile:
        import shutil
        for d in ("/tmp/cova_prof1", "/tmp/cova_prof2"):
            shutil.rmtree(d, ignore_errors=True)
            os.makedirs(d)
        kw1 = dict(trace=True, tmpdir="/tmp/cova_prof1")
        kw2 = dict(trace=True, tmpdir="/tmp/cova_prof2")

    if "cova" not in _CACHE:
        _CACHE["cova"] = _build_cova_nc()
    cova_in = [{"x2p": np.ascontiguousarray(x2[WS * k : WS * (k + 1)])}
               for k in range(N_CORES)]
    res1 = run_bass_kernel_spmd(_CACHE["cova"], cova_in, core_ids, **kw1)
    cova = np.concatenate([res1.results[k]["cova_pair"] for k in range(N_CORES)], 0)

    chol = np.linalg.cholesky(cova.astype(np.float64)).astype(np.float32)
    mats = np.ascontiguousarray(np.concatenate([chol[:ACT_W], cova[ACT_W:]], 0))

    if "sim" not in _CACHE:
        _CACHE["sim"] = _build_sim_nc()
    sim_in = [{"x1s": np.ascontiguousarray(x1[BS * k : BS * (k + 1)]),
               "mats": mats} for k in range(N_CORES)]
    res2 = run_bass_kernel_spmd(_CACHE["sim"], sim_in, core_ids, **kw2)
    if profile:
        _CACHE["exec_ns"] = (res1.exec_time_ns, res2.exec_time_ns)
    sim = np.concatenate([res2.results[k]["sim"] for k in range(N_CORES)], 0)
    return sim.reshape(B, W * HW)
